# revision 1
# baseline (speedup 1.0000x reference)
"""HSIC pairwise loss kernel for trn2 (8 NeuronCores).

Math: reference builds K_c = (w^2 w^2T) * (E_c E_c^T), M_c = R K_c, and sums
tr(M_i M_j) over i<j. With F_c = w^2 * E_c (row scaling), R the centering
matrix (idempotent):
    tr(R K_i R K_j) = ||G_i^T G_j||_F^2,  G_c = F_c - colmean(F_c)
and with A_ij = F_i^T F_j, s_c = F_c^T 1:
    G_i^T G_j = A_ij - (1/n) s_i s_j^T
so loss = sum_{i<j} ||A_ij - s_i s_j^T / n||_F^2 / (n-1)^2.

Device work: the 45 A_ij blocks [256,256] (contraction over n=4096) at
half-chunk granularity (20 units of 128 cols). Each core loads 9 units
(18.9 MB) and computes 8 matmul windows per k-tile into 8 PSUM banks,
fp32 data issued as float32r (full-rate on the PE at N>=256). A ones
column folded into the moving layout yields the column sums s for free.
Host assembles quadrants, applies the rank-1 centering correction and the
final scalar reduction in float64 (~3 MFLOP).
"""

import numpy as np
from contextlib import ExitStack

import concourse.bass as bass
import concourse.tile as tile
from concourse import bacc, mybir
from concourse import bass_utils

N = 4096
KT = 32            # k tiles of 128 rows
UNITS = 9          # half-chunk units per core
DCOLS = UNITS * 128          # 1152 data cols per k-tile
ROW = DCOLS + 2              # + two ones columns (fp32r needs even N)

# 8 cores x 9 units (of 20 half-chunks); covers all 180 cross-parent
# half-pairs via the fixed window pattern below (found by search).
ASSIGN = [
    [0, 19, 9, 4, 3, 18, 13, 17, 1],
    [10, 5, 8, 18, 4, 12, 9, 16, 15],
    [5, 17, 12, 9, 18, 2, 7, 14, 11],
    [14, 7, 12, 11, 2, 13, 1, 15, 16],
    [0, 17, 3, 19, 4, 6, 12, 11, 15],
    [14, 2, 9, 16, 5, 3, 19, 0, 6],
    [17, 18, 13, 11, 1, 6, 5, 8, 10],
    [3, 2, 19, 0, 14, 4, 7, 10, 8],
]

# (stat_slot, moving_start_col, n_cols). Moving cols 640..1151 are slots
# 5..8; col 1152 is the ones column (windows ending there also yield s).
WINDOWS = [
    (0, 640, 512),
    (1, 640, 512),
    (2, 640, 512),
    (3, 640, 512),
    (4, 640, 512),
    (5, 768, 386),
    (6, 896, 258),
    (7, 1024, 130),
]
OUT_COLS = sum(w[2] for w in WINDOWS)   # 3331

_CACHE = {}


def _build():
    f32 = mybir.dt.float32
    f32r = mybir.dt.float32r
    nc = bacc.Bacc("TRN2", target_bir_lowering=False, debug=False,
                   num_devices=8)
    x = nc.dram_tensor("x", [N, DCOLS], f32, kind="ExternalInput").ap()
    w = nc.dram_tensor("w", [128, KT], f32, kind="ExternalInput").ap()
    out = nc.dram_tensor("out", [128, OUT_COLS], f32,
                         kind="ExternalOutput").ap()

    with tile.TileContext(nc) as tc:
        with ExitStack() as ctx:
            spool = ctx.enter_context(tc.tile_pool(name="sw", bufs=1))
            xpool = ctx.enter_context(tc.tile_pool(name="xs", bufs=4))
            fpool = ctx.enter_context(tc.tile_pool(name="f", bufs=KT))
            psum = ctx.enter_context(tc.tile_pool(name="ps", bufs=1,
                                                  space="PSUM"))
            opool = ctx.enter_context(tc.tile_pool(name="o", bufs=1))

            swt = spool.tile([128, KT], f32, tag="swraw")
            nc.sync.dma_start(swt[:], w)
            sw2 = spool.tile([128, KT], f32, tag="swsq")
            nc.scalar.square(sw2[:], swt[:])
            one = spool.tile([128, 2], f32, tag="one")
            nc.vector.memset(one[:], 1.0)

            ps = []
            for i, (_, _, nw) in enumerate(WINDOWS):
                pst = psum.tile([128, nw], f32, tag=f"ps{i}", name=f"ps{i}")
                ps.append(pst)

            for k in range(KT):
                xr = xpool.tile([128, DCOLS], f32)
                nc.sync.dma_start(xr[:], x[k * 128:(k + 1) * 128, :])
                ft = fpool.tile([128, ROW], f32r)
                nc.vector.tensor_copy(ft[:, DCOLS:ROW], one[:])
                nc.scalar.mul(ft[:, 0:DCOLS], xr[:], sw2[:, k:k + 1])
                for wi, (s, mc, nw) in enumerate(WINDOWS):
                    nc.tensor.matmul(
                        ps[wi][:, 0:nw],
                        ft[:, s * 128:(s + 1) * 128],
                        ft[:, mc:mc + nw],
                        start=(k == 0),
                        stop=(k == KT - 1),
                    )

            ot = opool.tile([128, OUT_COLS], f32)
            col = 0
            for wi, (s, mc, nw) in enumerate(WINDOWS):
                eng = nc.vector if wi % 2 == 0 else nc.scalar
                if eng is nc.vector:
                    eng.tensor_copy(ot[:, col:col + nw], ps[wi][:, 0:nw])
                else:
                    eng.copy(ot[:, col:col + nw], ps[wi][:, 0:nw])
                col += nw
            nc.sync.dma_start(out, ot[:])
    nc.compile()
    return nc


def _get_nc():
    if "nc" not in _CACHE:
        _CACHE["nc"] = _build()
    return _CACHE["nc"]


def _in_maps(X, w):
    wt = np.ascontiguousarray(w.reshape(KT, 128).T)
    maps = []
    for units in ASSIGN:
        xc = np.concatenate([X[:, u * 128:(u + 1) * 128] for u in units],
                            axis=1)
        maps.append({"x": np.ascontiguousarray(xc), "w": wt})
    return maps


def _assemble(outs):
    quad = {}
    svec = {}
    for c, units in enumerate(ASSIGN):
        o = outs[c].astype(np.float64)
        col = 0
        for (s, mc, nw) in WINDOWS:
            su = units[s]
            block = o[:, col:col + nw]
            col += nw
            m0 = mc // 128
            for t in range((nw - (2 if nw % 128 else 0)) // 128):
                quad[(su, units[m0 + t])] = block[:, t * 128:(t + 1) * 128]
            if nw % 128:
                svec[su] = block[:, nw - 2]
    loss = 0.0
    for i in range(10):
        s_i = np.concatenate([svec[2 * i], svec[2 * i + 1]])
        for j in range(i + 1, 10):
            s_j = np.concatenate([svec[2 * j], svec[2 * j + 1]])
            A = np.empty((256, 256))
            for a in range(2):
                for b in range(2):
                    u, v = 2 * i + a, 2 * j + b
                    q = quad[(u, v)] if (u, v) in quad else quad[(v, u)].T
                    A[a * 128:(a + 1) * 128, b * 128:(b + 1) * 128] = q
            C = A - np.outer(s_i, s_j) / float(N)
            loss += float((C * C).sum())
    loss /= float((N - 1) * (N - 1))
    return np.asarray([loss], np.float32)


def kernel(final_readout, weight, _trace=False):
    X = np.ascontiguousarray(np.asarray(final_readout, np.float32))
    w = np.asarray(weight, np.float32)
    nc = _get_nc()
    res = bass_utils.run_bass_kernel_spmd(
        nc, _in_maps(X, w), core_ids=list(range(8)), trace=_trace)
    _CACHE["last_results"] = res
    return _assemble([r["out"] for r in res.results])



# revision 8
# speedup vs baseline: 1.7693x; 1.7693x over previous
"""HSIC pairwise loss kernel for trn2 (8 NeuronCores), fp8 DoubleRow version.

Math: with F_c = w^2 * E_c (row scaling), R the centering matrix:
    tr(R K_i R K_j) = ||G_i^T G_j||_F^2,  G_c = F_c - colmean(F_c)
and with A_ij = F_i^T F_j, s_c = F_c^T 1, u_c = F_c s_c:
    ||G_i^T G_j||^2 = ||A_ij||^2 - 2 u_i.u_j / n + ||s_i||^2 ||s_j||^2 / n^2
so only the 45 ||A_ij||_F^2 scalars need the O(n d^2) contraction; the
corrections are O(n d) and run on host in float64.

Device: inputs are host-converted to fp8e4m3 (loss error ~1.5e-3, gate 2e-2).
Each core computes 6 of the 45 A-blocks via a fixed "shape" of 6 edges over
5 chunk slots (SPMD: one program, per-core chunk->slot data mapping; 8x6=48
instances cover all 45 pairs, 3 duplicates dropped on host). Contraction
over n=4096 runs as 16 k-steps of 256 rows (DoubleRow, 2 fp8 rows/cycle) or
32 k-steps of 128 rows (plain fp8). PSUM accumulates A-blocks; on-device
square+reduce produces per-(edge,half) scalars [128, 12]; host does the
final cross-partition sum and corrections.
"""

import numpy as np
import ml_dtypes
from contextlib import ExitStack

import concourse.bass as bass
import concourse.tile as tile
from concourse import bacc, mybir
from concourse import bass_utils

N = 4096
EMB = 256
KP = 16              # k-steps of 256 rows (DoubleRow)
KPLAIN = 32          # k-steps of 128 rows (plain)
C = 5 * EMB          # 1280 data cols per k-group (5 chunk slots)

# Shape P*: slots layout [b,c,d,e,a]; claws a x {b,c}, b x {d,e}, c x {d,e}
# Shape D3: slots layout [b,c,d,e,a]; claws a x {b,c,d}, b x {c,d,e}
# Slot col offsets: b=0, c=256, d=512, e=768, a=1024.
import os as _os
SHAPE = _os.environ.get("HSIC_SHAPE", "PSTAR")   # "PSTAR" | "D3"
USE_DR = _os.environ.get("HSIC_DR", "1") == "1"  # off = plain fp8, 32 k-steps
DEDUP_LDW = False    # set inst.ldweights=False on repeated-stationary MMs
WARMUP_MM = int(_os.environ.get("HSIC_WARMUP", "8"))

# 8 instances (a,b,c,d,e) covering all 45 chunk pairs (3 dup edges).
TUPLES_PSTAR = [
    (3, 1, 5, 2, 7), (6, 2, 0, 8, 7), (5, 1, 8, 4, 6), (2, 4, 3, 6, 9),
    (3, 0, 7, 9, 4), (5, 9, 0, 2, 1), (9, 6, 8, 7, 1), (6, 3, 5, 4, 8),
]
TUPLES_D3 = [
    (1, 4, 2, 9, 7), (3, 5, 1, 7, 8), (8, 7, 6, 2, 1), (9, 0, 2, 7, 6),
    (3, 9, 6, 8, 5), (8, 0, 1, 4, 5), (5, 6, 2, 4, 1), (3, 2, 0, 4, 6),
]

# Units: (stat_slot_off, stat_half, mov_off, mov_w, psum_tag, psum_col)
# ordered so consecutive units share the stationary where possible.
# Edge order within the 12 accum scalars is fixed per shape (host decodes).
A_OFF = 4 * EMB      # slot a


def _units():
    us = []
    if SHAPE == "PSTAR":
        # stat a: mov [b,c] 0:512 ; stat b: mov [d,e] 512:1024 ;
        # stat c: mov [d,e] 512:1024
        for h in range(2):
            us.append((A_OFF + h * 128, 0, 512, f"pA{h}"))
        for h in range(2):
            us.append((0 + h * 128, 512, 512, f"pB{h}"))
        for h in range(2):
            us.append((EMB + h * 128, 512, 512, f"pC{h}"))
    else:  # D3
        # stat a: mov [b,c,d] 0:768 ; stat b: mov [c,d,e] 256:1024
        for h in range(2):
            us.append((A_OFF + h * 128, 0, 512, f"pA{h}"))
            us.append((A_OFF + h * 128, 512, 256, f"pAx{h}"))
        for h in range(2):
            us.append((0 + h * 128, 256, 512, f"pB{h}"))
            us.append((0 + h * 128, 768, 256, f"pBx{h}"))
    return us


def _edges_of(t):
    a, b, c, d, e = t
    if SHAPE == "PSTAR":
        return [(a, b), (a, c), (b, d), (b, e), (c, d), (c, e)]
    return [(a, b), (a, c), (a, d), (b, c), (b, d), (b, e)]


def _scalar_cols():
    """Map accum-scalar column -> (edge_index, half). Edge indices follow
    _edges_of order."""
    cols = []
    if SHAPE == "PSTAR":
        # units: aH0 [ab|ac], aH1, bH0 [bd|be], bH1, cH0 [cd|ce], cH1
        for h in range(2):
            cols += [(0, h), (1, h)]
        for h in range(2):
            cols += [(2, h), (3, h)]
        for h in range(2):
            cols += [(4, h), (5, h)]
    else:
        # units per half: a: [ab|ac] + [ad] ; b: [bc|bd] + [be]
        for h in range(2):
            cols += [(0, h), (1, h), (2, h)]
        for h in range(2):
            cols += [(3, h), (4, h), (5, h)]
    return cols

NSCAL = 12

_CACHE = {}


def _build():
    f32 = mybir.dt.float32
    f8 = mybir.dt.float8e4
    DR = mybir.MatmulPerfMode.DoubleRow
    nc = bacc.Bacc("TRN2", target_bir_lowering=False, debug=False,
                   num_devices=8)
    if USE_DR:
        x = nc.dram_tensor("x", [KP * 128, 2 * C], f8,
                           kind="ExternalInput").ap()
    else:
        x = nc.dram_tensor("x", [N, C], f8, kind="ExternalInput").ap()
    out = nc.dram_tensor("out", [128, NSCAL], f32,
                         kind="ExternalOutput").ap()

    units = _units()
    nk = KP if USE_DR else KPLAIN

    with tile.TileContext(nc) as tc:
        with ExitStack() as ctx:
            spool = ctx.enter_context(tc.tile_pool(name="sw", bufs=1))
            xpool = ctx.enter_context(tc.tile_pool(name="xs", bufs=4))
            psum = ctx.enter_context(tc.tile_pool(name="ps", bufs=1,
                                                  space="PSUM"))
            opool = ctx.enter_context(tc.tile_pool(name="o", bufs=1))

            pst = {}
            for (so, mo, mw, tag) in units:
                if tag not in pst:
                    pst[tag] = psum.tile([128, mw], f32, tag=tag, name=tag)

            # PE warmup on constant tile (keeps HAM awake during first DMAs);
            # writes the first unit's psum tile, overwritten by real k=0 MMs.
            if WARMUP_MM:
                wt = spool.tile([128, 2, 512], f8, tag="warm")
                nc.vector.memset(wt[:], 1.0)
                wps = pst[units[0][3]]
                ww = min(512, units[0][2])
                for i in range(WARMUP_MM):
                    if USE_DR:
                        nc.tensor.matmul(wps[:, 0:ww], wt[:, :, 0:128],
                                         wt[:, :, 0:ww], start=True,
                                         stop=True, perf_mode=DR,
                                         skip_group_check=True)
                    else:
                        nc.tensor.matmul(wps[:, 0:ww], wt[:, 0, 0:128],
                                         wt[:, 0, 0:ww], start=True,
                                         stop=True, skip_group_check=True)
            # scratch for reduce stage + ACT table warmup
            sscr = spool.tile([128, 512], f32, tag="sscr")
            acc = spool.tile([128, NSCAL], f32, tag="acc")
            nc.vector.memset(acc[:], 0.0)
            nc.scalar.activation(sscr[:, 0:2], acc[:, 0:2],
                                 mybir.ActivationFunctionType.Square)

            for k in range(nk):
                if USE_DR:
                    xr = xpool.tile([128, 2, C], f8)
                    nc.sync.dma_start(xr[:], x[k * 128:(k + 1) * 128, :])
                else:
                    xr = xpool.tile([128, C], f8)
                    nc.sync.dma_start(xr[:], x[k * 128:(k + 1) * 128, :])
                prev_stat = None
                for (so, mo, mw, tag) in units:
                    if USE_DR:
                        inst = nc.tensor.matmul(
                            pst[tag][:, 0:mw],
                            xr[:, :, so:so + 128],
                            xr[:, :, mo:mo + mw],
                            start=(k == 0), stop=(k == nk - 1),
                            perf_mode=DR)
                    else:
                        inst = nc.tensor.matmul(
                            pst[tag][:, 0:mw],
                            xr[:, so:so + 128],
                            xr[:, mo:mo + mw],
                            start=(k == 0), stop=(k == nk - 1))
                    if DEDUP_LDW and prev_stat == so:
                        inst.ldweights = False
                    prev_stat = so

            # square+reduce each 256-col block into acc[:, j]
            jcol = 0
            blocks = []
            for (so, mo, mw, tag) in units:
                for b in range(mw // 256):
                    blocks.append((pst[tag], b * 256))
            assert len(blocks) == NSCAL
            vec_share = int(_os.environ.get("HSIC_VEC_REDUCE", "4"))
            vscr = spool.tile([128, 1024], f32, tag="vscr")
            for j, (pt, off) in enumerate(blocks):
                if j < NSCAL - vec_share:
                    nc.scalar.activation(
                        sscr[:, 0:256], pt[:, off:off + 256],
                        mybir.ActivationFunctionType.Square,
                        accum_out=acc[:, j:j + 1])
                else:
                    # DVE can read only one PSUM operand: copy to SBUF, then
                    # square-reduce SBUF x SBUF.
                    vo = 256 * (j % 2)
                    nc.vector.tensor_copy(vscr[:, vo:vo + 256],
                                          pt[:, off:off + 256])
                    nc.vector.tensor_tensor_reduce(
                        out=vscr[:, 512 + vo:768 + vo],
                        in0=vscr[:, vo:vo + 256],
                        in1=vscr[:, vo:vo + 256],
                        scale=1.0, scalar=0.0,
                        op0=mybir.AluOpType.mult,
                        op1=mybir.AluOpType.add,
                        accum_out=acc[:, j:j + 1])
            nc.sync.dma_start(out, acc[:])
    nc.compile()
    return nc


def _get_nc():
    if "nc" not in _CACHE:
        _CACHE["nc"] = _build()
    return _CACHE["nc"]


def _prep_inputs(F8):
    """F8: [N, 2560] fp8 array (already scaled). Returns per-core in_maps."""
    tuples = TUPLES_PSTAR if SHAPE == "PSTAR" else TUPLES_D3
    maps = []
    for t in tuples:
        a, b, c, d, e = t
        order = [b, c, d, e, a]
        xc = np.concatenate(
            [F8[:, u * EMB:(u + 1) * EMB] for u in order], axis=1)
        if USE_DR:
            xc = np.ascontiguousarray(
                xc.reshape(KP, 2, 128, C).transpose(0, 2, 1, 3)
                  .reshape(KP * 128, 2 * C))
        else:
            xc = np.ascontiguousarray(xc)
        maps.append({"x": xc})
    return maps


def kernel(final_readout, weight, _trace=False):
    X = np.asarray(final_readout, np.float32)
    w = np.asarray(weight, np.float32)
    F64 = (w.astype(np.float64) ** 2) * X.astype(np.float64)
    F32 = F64.astype(np.float32)

    # power-of-2 scale into fp8 sweet spot
    mx = float(np.abs(F32).max())
    gamma = 2.0 ** int(np.clip(np.floor(np.log2(100.0 / mx)) if mx > 0
                               else 0, -30, 30))
    F8 = (F32 * np.float32(gamma)).astype(ml_dtypes.float8_e4m3)

    nc = _get_nc()
    import os
    if os.environ.get("BASS_KERNEL_SIM"):
        from concourse.bass_interp import CoreSim
        results = []
        for im in _prep_inputs(F8):
            sim = CoreSim(nc, trace=False)
            sim.tensor("x")[:] = im["x"]
            sim.simulate(check_with_hw=False)
            results.append({"out": np.array(sim.tensor("out"))})
        res = bass_utils.BassKernelResults(
            results=results, instructions_and_trace=None,
            profile_json=None, exec_time_ns=None)
    else:
        res = bass_utils.run_bass_kernel_spmd(
            nc, _prep_inputs(F8), core_ids=list(range(8)), trace=_trace)
    _CACHE["last_results"] = res

    # host: edge sums (dedup), corrections in float64
    tuples = TUPLES_PSTAR if SHAPE == "PSTAR" else TUPLES_D3
    scol = _scalar_cols()
    edge_sq = {}
    for ci, t in enumerate(tuples):
        acc = res.results[ci]["out"].astype(np.float64)  # [128, 12]
        edges = _edges_of(t)
        esum = {}
        for j, (ei, h) in enumerate(scol):
            key = tuple(sorted(edges[ei]))
            esum[key] = esum.get(key, 0.0) + float(acc[:, j].sum())
        for key, v in esum.items():
            if key not in edge_sq:
                edge_sq[key] = v
    g4 = gamma ** 4

    s = F64.sum(axis=0)                        # [2560]
    loss = 0.0
    for i in range(10):
        si = s[i * EMB:(i + 1) * EMB]
        ui = F64[:, i * EMB:(i + 1) * EMB] @ si
        for j in range(i + 1, 10):
            sj = s[j * EMB:(j + 1) * EMB]
            uj = F64[:, j * EMB:(j + 1) * EMB] @ sj
            a2 = edge_sq[(i, j)] / g4
            loss += a2 - 2.0 / N * float(ui @ uj) \
                + float(si @ si) * float(sj @ sj) / (N * N)
    loss /= float((N - 1) * (N - 1))
    return np.asarray([loss], np.float32)


# revision 16
# speedup vs baseline: 1.7907x; 1.0121x over previous
"""HSIC pairwise loss kernel for trn2 (8 NeuronCores), fp8 DoubleRow version.

Math: with F_c = w^2 * E_c (row scaling), R the centering matrix:
    tr(R K_i R K_j) = ||G_i^T G_j||_F^2,  G_c = F_c - colmean(F_c)
and with A_ij = F_i^T F_j, s_c = F_c^T 1, u_c = F_c s_c:
    ||G_i^T G_j||^2 = ||A_ij||^2 - 2 u_i.u_j / n + ||s_i||^2 ||s_j||^2 / n^2
so only the 45 ||A_ij||_F^2 scalars need the O(n d^2) contraction; the
corrections are O(n d) and run on host in float64.

Device: inputs are host-converted to fp8e4m3 (loss error ~1.5e-3 vs the 2e-2
gate). Uniform SPMD program: every core runs the same 6-edge "claw" shape
P* = {ab, ac, bd, be, cd, ce} over 5 chunk slots; the per-core chunk->slot
mapping makes the 8x6=48 edge instances cover all 45 chunk pairs (the 3
statically-known duplicates are recomputed and subtracted on the host).
The contraction over n=4096 runs as 16 k-steps of 256 rows each
(MatmulPerfMode.DoubleRow processes 2 fp8 rows/cycle: measured 215.8 ns per
512-col matmul, i.e. the 512-cycle floor). A-blocks accumulate in two wide
PSUM tiles; a short on-device square+reduce (scalar ACT x2 + vector
copy+reduce, in parallel) emits 3 partial scalars per partition; the host
does the final O(1) assembly in float64.
"""

import os as _os

import numpy as np
import ml_dtypes
from contextlib import ExitStack

import concourse.bass as bass
import concourse.tile as tile
from concourse import bacc, mybir
from concourse import bass_utils

N = 4096
EMB = 256
KP = 16              # k-steps of 256 rows (DoubleRow)
C = 5 * EMB          # 1280 data cols per k-group (5 chunk slots)
NSCAL = 8            # accum scalars per core (6 used)
WARMUP_MM = int(_os.environ.get("HSIC_WARMUP", "8"))

# Shape P*: slot layout [b,c,d,e,a] with col offsets b=0, c=256, d=512,
# e=768, a=1024; claws a x {b,c}, b x {d,e}, c x {d,e}.
A_OFF = 4 * EMB

# 8 instances (a,b,c,d,e) covering all 45 chunk pairs (3 dup edges).
TUPLES = [
    (3, 1, 5, 2, 7), (6, 2, 0, 8, 7), (5, 1, 8, 4, 6), (2, 4, 3, 6, 9),
    (3, 0, 7, 9, 4), (5, 9, 0, 2, 1), (9, 6, 8, 7, 1), (6, 3, 5, 4, 8),
]


def _edges_of(t):
    a, b, c, d, e = t
    return [(a, b), (a, c), (b, d), (b, e), (c, d), (c, e)]


def _edge_mult():
    mult = {}
    for t in TUPLES:
        for e in _edges_of(t):
            key = tuple(sorted(e))
            mult[key] = mult.get(key, 0) + 1
    return mult


# units: (stat_col, mov_col); all 512-out DoubleRow matmuls
UNITS = [(A_OFF, 0), (A_OFF + 128, 0),
         (0, 512), (128, 512),
         (EMB, 512), (EMB + 128, 512)]

_CACHE = {}


def _build():
    f32 = mybir.dt.float32
    f8 = mybir.dt.float8e4
    DR = mybir.MatmulPerfMode.DoubleRow
    nc = bacc.Bacc("TRN2", target_bir_lowering=False, debug=False,
                   num_devices=8)
    x = nc.dram_tensor("x", [KP * 128, 2 * C], f8, kind="ExternalInput").ap()
    out = nc.dram_tensor("out", [128, NSCAL], f32,
                         kind="ExternalOutput").ap()

    with tile.TileContext(nc) as tc:
        with ExitStack() as ctx:
            spool = ctx.enter_context(tc.tile_pool(name="sw", bufs=1))
            xpool = ctx.enter_context(tc.tile_pool(name="xs", bufs=int(_os.environ.get("HSIC_XBUFS", "4"))))
            psum = ctx.enter_context(tc.tile_pool(name="ps", bufs=1,
                                                  space="PSUM"))

            # six single-bank PSUM tiles (one per unit; cross-bank PSUM
            # access hangs the device)
            pts = [psum.tile([128, 512], f32, tag=f"pb{i}", name=f"pb{i}")
                   for i in range(6)]
            pslice = [(pts[i], 0) for i in range(6)]
            p1 = pts[0]

            # PE warmup on constant data: burns the HAM cold phase while the
            # first input tiles stream in. Overwritten by the real k=0 MMs.
            if WARMUP_MM:
                wt = spool.tile([128, 2, 512], f8, tag="warm")
                nc.vector.memset(wt[:], 1.0)
                for i in range(WARMUP_MM):
                    nc.tensor.matmul(p1[:, 0:512], wt[:, :, 0:128],
                                     wt[:, :, 0:512], start=True,
                                     stop=True, perf_mode=DR,
                                     skip_group_check=True)
            # scratch for reduce stage + ACT table preload
            sscr = spool.tile([128, 512], f32, tag="sscr")
            acc = spool.tile([128, NSCAL], f32, tag="acc")
            nc.vector.memset(acc[:], 0.0)
            nc.scalar.activation(sscr[:, 0:2], acc[:, 0:2],
                                 mybir.ActivationFunctionType.Square)

            for k in range(KP):
                xr = xpool.tile([128, 2, C], f8)
                nc.sync.dma_start(xr[:], x[k * 128:(k + 1) * 128, :])
                for ui, (so, mo) in enumerate(UNITS):
                    pt, poff = pslice[ui]
                    nc.tensor.matmul(
                        pt[:, poff:poff + 512],
                        xr[:, :, so:so + 128],
                        xr[:, :, mo:mo + 512],
                        start=(k == 0), stop=(k == KP - 1),
                        perf_mode=DR)

            # reduce: scalar ACT squares all six banks (vector TTR path
            # fails at runtime on hw)
            for i in range(6):
                nc.scalar.activation(
                    sscr[:, 0:512], pts[i][:, 0:512],
                    mybir.ActivationFunctionType.Square,
                    accum_out=acc[:, i:i + 1])
            nc.sync.dma_start(out, acc[:])
    nc.compile()
    return nc


def _get_nc():
    if "nc" not in _CACHE:
        _CACHE["nc"] = _build()
    return _CACHE["nc"]


def _prep_inputs(F8):
    """F8: [N, 2560] fp8 array (already scaled). Returns per-core in_maps
    with the k-pair row interleave the DoubleRow APs expect."""
    maps = []
    for t in TUPLES:
        a, b, c, d, e = t
        order = [b, c, d, e, a]
        xc = np.concatenate(
            [F8[:, u * EMB:(u + 1) * EMB] for u in order], axis=1)
        xc = np.ascontiguousarray(
            xc.reshape(KP, 2, 128, C).transpose(0, 2, 1, 3)
              .reshape(KP * 128, 2 * C))
        maps.append({"x": xc})
    return maps


def kernel(final_readout, weight, _trace=False):
    X = np.asarray(final_readout, np.float32)
    w = np.asarray(weight, np.float32)
    F64 = (w.astype(np.float64) ** 2) * X.astype(np.float64)
    F32 = F64.astype(np.float32)

    # power-of-2 scale into the fp8 sweet spot (exact to undo)
    mx = float(np.abs(F32).max())
    gamma = 2.0 ** int(np.clip(np.floor(np.log2(100.0 / mx)) if mx > 0
                               else 0, -30, 30))
    F8 = (F32 * np.float32(gamma)).astype(ml_dtypes.float8_e4m3)

    nc = _get_nc()
    if _os.environ.get("BASS_KERNEL_SIM"):
        from concourse.bass_interp import CoreSim
        results = []
        for im in _prep_inputs(F8):
            sim = CoreSim(nc, trace=False)
            sim.tensor("x")[:] = im["x"]
            sim.simulate(check_with_hw=False)
            results.append({"out": np.array(sim.tensor("out"))})
        res = bass_utils.BassKernelResults(
            results=results, instructions_and_trace=None,
            profile_json=None, exec_time_ns=None)
    else:
        res = bass_utils.run_bass_kernel_spmd(
            nc, _prep_inputs(F8), core_ids=list(range(8)), trace=_trace)
    _CACHE["last_results"] = res

    # device total of ||A8_ij||^2 over all 48 instances (scaled by gamma^4)
    acc_sum = 0.0
    for ci in range(8):
        acc_sum += float(res.results[ci]["out"].astype(np.float64).sum())

    # subtract the duplicate instances (identical fp8 data -> host fp32
    # recompute matches the device value to ~1e-7)
    F8f = F8.astype(np.float32)
    for (i, j), m in _edge_mult().items():
        if m > 1:
            a8 = F8f[:, i * EMB:(i + 1) * EMB].T @ \
                 F8f[:, j * EMB:(j + 1) * EMB]
            acc_sum -= (m - 1) * float((a8.astype(np.float64) ** 2).sum())
    total_sq = acc_sum / (gamma ** 4)

    # exact corrections in float64
    s = F64.sum(axis=0)
    loss = total_sq
    for i in range(10):
        si = s[i * EMB:(i + 1) * EMB]
        ui = F64[:, i * EMB:(i + 1) * EMB] @ si
        for j in range(i + 1, 10):
            sj = s[j * EMB:(j + 1) * EMB]
            uj = F64[:, j * EMB:(j + 1) * EMB] @ sj
            loss += -2.0 / N * float(ui @ uj) \
                + float(si @ si) * float(sj @ sj) / (N * N)
    loss /= float((N - 1) * (N - 1))
    return np.asarray([loss], np.float32)


# revision 17
# speedup vs baseline: 1.9238x; 1.0744x over previous
"""HSIC pairwise loss kernel for trn2 (8 NeuronCores), fp8 DoubleRow version.

Math: with F_c = w^2 * E_c (row scaling), R the centering matrix:
    tr(R K_i R K_j) = ||G_i^T G_j||_F^2,  G_c = F_c - colmean(F_c)
and with A_ij = F_i^T F_j, s_c = F_c^T 1, u_c = F_c s_c:
    ||G_i^T G_j||^2 = ||A_ij||^2 - 2 u_i.u_j / n + ||s_i||^2 ||s_j||^2 / n^2
so only the 45 ||A_ij||_F^2 scalars need the O(n d^2) contraction; the
corrections are O(n d) and run on host in float64.

Device: inputs are host-converted to fp8e4m3 (loss error ~1.5e-3 vs the 2e-2
gate). Uniform SPMD program: every core runs the same 6-edge "claw" shape
P* = {ab, ac, bd, be, cd, ce} over 5 chunk slots; the per-core chunk->slot
mapping makes the 8x6=48 edge instances cover all 45 chunk pairs (the 3
statically-known duplicates are recomputed and subtracted on the host).
The contraction over n=4096 runs as 16 k-steps of 256 rows each
(MatmulPerfMode.DoubleRow processes 2 fp8 rows/cycle: measured 215.8 ns per
512-col matmul, i.e. the 512-cycle floor). A-blocks accumulate in two wide
PSUM tiles; a short on-device square+reduce (scalar ACT x2 + vector
copy+reduce, in parallel) emits 3 partial scalars per partition; the host
does the final O(1) assembly in float64.
"""

import os as _os

import numpy as np
import ml_dtypes
from contextlib import ExitStack

import concourse.bass as bass
import concourse.tile as tile
from concourse import bacc, mybir
from concourse import bass_utils

N = 4096
EMB = 256
KP = 16              # k-steps of 256 rows (DoubleRow)
C = 5 * EMB          # 1280 data cols per k-group (5 chunk slots)
NSCAL = 8            # accum scalars per core (6 used)
WARMUP_MM = int(_os.environ.get("HSIC_WARMUP", "8"))

# Shape P*: slot layout [b,c,d,e,a] with col offsets b=0, c=256, d=512,
# e=768, a=1024; claws a x {b,c}, b x {d,e}, c x {d,e}.
A_OFF = 4 * EMB

# 8 instances (a,b,c,d,e) covering all 45 chunk pairs (3 dup edges).
TUPLES = [
    (3, 1, 5, 2, 7), (6, 2, 0, 8, 7), (5, 1, 8, 4, 6), (2, 4, 3, 6, 9),
    (3, 0, 7, 9, 4), (5, 9, 0, 2, 1), (9, 6, 8, 7, 1), (6, 3, 5, 4, 8),
]


def _edges_of(t):
    a, b, c, d, e = t
    return [(a, b), (a, c), (b, d), (b, e), (c, d), (c, e)]


def _edge_mult():
    mult = {}
    for t in TUPLES:
        for e in _edges_of(t):
            key = tuple(sorted(e))
            mult[key] = mult.get(key, 0) + 1
    return mult


# units: (stat_col, mov_col); all 512-out DoubleRow matmuls
UNITS = [(A_OFF, 0), (A_OFF + 128, 0),
         (0, 512), (128, 512),
         (EMB, 512), (EMB + 128, 512)]

_CACHE = {}


def _build():
    f32 = mybir.dt.float32
    f8 = mybir.dt.float8e4
    DR = mybir.MatmulPerfMode.DoubleRow
    nc = bacc.Bacc("TRN2", target_bir_lowering=False, debug=False,
                   num_devices=8)
    x = nc.dram_tensor("x", [KP * 128, 2 * C], f8, kind="ExternalInput").ap()
    out = nc.dram_tensor("out", [128, NSCAL], f32,
                         kind="ExternalOutput").ap()

    with tile.TileContext(nc) as tc:
        with ExitStack() as ctx:
            spool = ctx.enter_context(tc.tile_pool(name="sw", bufs=1))
            xpool = ctx.enter_context(tc.tile_pool(name="xs", bufs=int(_os.environ.get("HSIC_XBUFS", "16"))))
            psum = ctx.enter_context(tc.tile_pool(name="ps", bufs=1,
                                                  space="PSUM"))

            # six single-bank PSUM tiles (one per unit; cross-bank PSUM
            # access hangs the device)
            pts = [psum.tile([128, 512], f32, tag=f"pb{i}", name=f"pb{i}")
                   for i in range(6)]
            pslice = [(pts[i], 0) for i in range(6)]
            p1 = pts[0]

            # PE warmup on constant data: burns the HAM cold phase while the
            # first input tiles stream in. Overwritten by the real k=0 MMs.
            if WARMUP_MM:
                wt = spool.tile([128, 2, 512], f8, tag="warm")
                nc.vector.memset(wt[:], 1.0)
                for i in range(WARMUP_MM):
                    nc.tensor.matmul(p1[:, 0:512], wt[:, :, 0:128],
                                     wt[:, :, 0:512], start=True,
                                     stop=True, perf_mode=DR,
                                     skip_group_check=True)
            # scratch for reduce stage + ACT table preload
            sscr = spool.tile([128, 512], f32, tag="sscr")
            acc = spool.tile([128, NSCAL], f32, tag="acc")
            nc.vector.memset(acc[:], 0.0)
            nc.scalar.activation(sscr[:, 0:2], acc[:, 0:2],
                                 mybir.ActivationFunctionType.Square)

            # issue all input-tile DMAs upfront (bufs=16: no recycling)
            xrs = []
            for k in range(KP):
                xr = xpool.tile([128, 2, C], f8)
                nc.sync.dma_start(xr[:], x[k * 128:(k + 1) * 128, :])
                xrs.append(xr)

            def mm(u, k):
                so, mo = UNITS[u]
                pt, poff = pslice[u]
                nc.tensor.matmul(
                    pt[:, poff:poff + 512],
                    xrs[k][:, :, so:so + 128],
                    xrs[k][:, :, mo:mo + 512],
                    start=(k == 0), stop=(k == KP - 1),
                    perf_mode=DR)

            # phase 1: k-outer (paced by the DMA stream)
            K1 = 7
            for k in range(K1):
                for u in range(6):
                    mm(u, k)
            # phase 2: unit-major; each bank completes early so its
            # square+reduce overlaps the next unit's matmuls
            for u in range(6):
                for k in range(K1, KP):
                    mm(u, k)
                nc.scalar.activation(
                    sscr[:, 0:512], pslice[u][0][:, 0:512],
                    mybir.ActivationFunctionType.Square,
                    accum_out=acc[:, u:u + 1])

            nc.sync.dma_start(out, acc[:])
    nc.compile()
    return nc


def _get_nc():
    if "nc" not in _CACHE:
        _CACHE["nc"] = _build()
    return _CACHE["nc"]


def _prep_inputs(F8):
    """F8: [N, 2560] fp8 array (already scaled). Returns per-core in_maps
    with the k-pair row interleave the DoubleRow APs expect."""
    maps = []
    for t in TUPLES:
        a, b, c, d, e = t
        order = [b, c, d, e, a]
        xc = np.concatenate(
            [F8[:, u * EMB:(u + 1) * EMB] for u in order], axis=1)
        xc = np.ascontiguousarray(
            xc.reshape(KP, 2, 128, C).transpose(0, 2, 1, 3)
              .reshape(KP * 128, 2 * C))
        maps.append({"x": xc})
    return maps


def kernel(final_readout, weight, _trace=False):
    X = np.asarray(final_readout, np.float32)
    w = np.asarray(weight, np.float32)
    F64 = (w.astype(np.float64) ** 2) * X.astype(np.float64)
    F32 = F64.astype(np.float32)

    # power-of-2 scale into the fp8 sweet spot (exact to undo)
    mx = float(np.abs(F32).max())
    gamma = 2.0 ** int(np.clip(np.floor(np.log2(100.0 / mx)) if mx > 0
                               else 0, -30, 30))
    F8 = (F32 * np.float32(gamma)).astype(ml_dtypes.float8_e4m3)

    nc = _get_nc()
    if _os.environ.get("BASS_KERNEL_SIM"):
        from concourse.bass_interp import CoreSim
        results = []
        for im in _prep_inputs(F8):
            sim = CoreSim(nc, trace=False)
            sim.tensor("x")[:] = im["x"]
            sim.simulate(check_with_hw=False)
            results.append({"out": np.array(sim.tensor("out"))})
        res = bass_utils.BassKernelResults(
            results=results, instructions_and_trace=None,
            profile_json=None, exec_time_ns=None)
    else:
        res = bass_utils.run_bass_kernel_spmd(
            nc, _prep_inputs(F8), core_ids=list(range(8)), trace=_trace)
    _CACHE["last_results"] = res

    # device total of ||A8_ij||^2 over all 48 instances (scaled by gamma^4)
    acc_sum = 0.0
    for ci in range(8):
        acc_sum += float(res.results[ci]["out"].astype(np.float64).sum())

    # subtract the duplicate instances (identical fp8 data -> host fp32
    # recompute matches the device value to ~1e-7)
    F8f = F8.astype(np.float32)
    for (i, j), m in _edge_mult().items():
        if m > 1:
            a8 = F8f[:, i * EMB:(i + 1) * EMB].T @ \
                 F8f[:, j * EMB:(j + 1) * EMB]
            acc_sum -= (m - 1) * float((a8.astype(np.float64) ** 2).sum())
    total_sq = acc_sum / (gamma ** 4)

    # exact corrections in float64
    s = F64.sum(axis=0)
    loss = total_sq
    for i in range(10):
        si = s[i * EMB:(i + 1) * EMB]
        ui = F64[:, i * EMB:(i + 1) * EMB] @ si
        for j in range(i + 1, 10):
            sj = s[j * EMB:(j + 1) * EMB]
            uj = F64[:, j * EMB:(j + 1) * EMB] @ sj
            loss += -2.0 / N * float(ui @ uj) \
                + float(si @ si) * float(sj @ sj) / (N * N)
    loss /= float((N - 1) * (N - 1))
    return np.asarray([loss], np.float32)
